# revision 48
# baseline (speedup 1.0000x reference)
"""Single-head causal attention (B=16, S=2048, d_model=384, d_q=64) on 8 trn2 cores.

Sharding: data-parallel over batch — 2 batches per core.

Per-core kernel design (bf16 matmul inputs, fp32 PSUM accumulation):
  - Both batches' attention streams are interleaved at score-block
    granularity so each stream's exp() latency (ACT engine) is hidden by
    the other stream's matmuls, and phase-A work (x load/transpose, QKV
    projection) is fed between attention blocks as deadline-markered
    filler with x DMAs issued one tile-group ahead.
  - x arrives via batched 4-tile DMAs (fewer, bigger descriptors keep
    the SP queue from throttling arrival rate) in A-priority order
    matched to the consumption schedule; tiles are cast to bf16 (ACT for
    batch 0, DVE for batch 1 — ACT is idle during the load phase) and
    PE-transposed with a PSUM->SBUF copyback.
  - Projections: Wq|Wk packed into one [128d, 128] stationary so Q^T and
    K^T [64, S] come from a single matmul stream.  V is computed
    naturally [s, 64] (lhsT = xT chunk stationary, rhs = Wv chunk) and
    augmented with a ones column -> V_aug [128, 65] per key block.
  - Scores in transposed layout: scoresT[k, q] = matmul(lhsT=K^T block
    [64, 128], rhs=Q^T [64, q]).  Full-width (512-col) score blocks are
    computed in pairs into a 2-bank PSUM tile so one 1024-wide exp()
    covers two blocks; diagonal blocks pack two-per-exp as well.  exp()
    has the 1/sqrt(d_q) scale folded in; no max-subtraction (scores are
    O(+-10), exp stays in fp32 range).
  - PV accumulates in NATURAL layout: out_nat[q, e'] += matmul(
    lhsT=P^T piece [128k, 128q] (stationary), rhs=V_aug [128k, 65]).
    Only 65 rows stream per piece (vs 512 transposed), the ones column
    yields the softmax denominator in col 64, and no output transpose is
    needed: epilogue is one strided reciprocal, one broadcast multiply,
    one DMA per panel.  A single start=True per panel opens the PSUM
    bank's pending-zero region, which initializes each q sub-block's
    first write (per-region start flags would re-arm the bank-wide
    region and corrupt already-accumulated sub-blocks).
  - Causal masking: block-diagonal coverage; the diagonal 128x128 block
    is masked after exp by zeroing q < k with gpsimd.affine_select.
  - HAM pacing: the hardware activity monitor halves the PE clock after
    ~11 consecutive 3.4us epochs of >=80% tensor activity.  Every 3rd
    round both streams emit PV immediately after exp (instead of the
    usual one-yield pending delay) and skip filler, injecting real PE
    idle that defers the throttle until the kernel is ending.
  - PSUM budget (8 banks): score pairs A/B (2+2), natural accumulators
    A/B (1+1), shared phase pool (2).
"""

import numpy as np

B, S, D, E = 16, 2048, 384, 64
N_CORES = 8
BPC = B // N_CORES  # batches per core
NB = S // 128  # 16 key blocks of 128
NCH = S // 512  # 4 projection chunks
H = 512  # attention panel width
NPAN = S // H  # 4 panels
SCALE = 1.0 / 8.0  # 1/sqrt(d_q)

_cache = {}


def _split_multi_waits(nc, max_waits=1):
    """Walrus codegen on this image rejects instructions carrying more than
    one sync wait (setupSyncWait: 'Too many sync wait commands').  Engines
    execute their queue in order, so excess waits can be moved onto NOP
    instructions inserted immediately before the owning instruction."""
    import concourse.mybir as mybir

    k = 0
    for f in nc.m.functions:
        for bb in f.blocks:
            insts = bb.instructions
            out = []
            changed = False
            for ins in insts:
                si = getattr(ins, "sync_info", None)
                waits = list(si.on_wait) if si is not None else []
                if len(waits) > max_waits:
                    changed = True
                    for extra in waits[:-max_waits]:
                        nop = mybir.InstNoOp(
                            name=f"wsplit-{k}", ins=[], outs=[]
                        )
                        k += 1
                        nop.engine = ins.engine
                        nop.sync_info = mybir.SyncInfo(
                            on_wait=[extra], on_update=[]
                        )
                        out.append(nop)
                    ins.sync_info = mybir.SyncInfo(
                        on_wait=waits[-max_waits:],
                        on_update=list(si.on_update),
                    )
                out.append(ins)
            if changed:
                bb.instructions = out


def _install_patches():
    """Register the NTFF profile hook so trace=True works under axon."""
    import sys
    import types

    if "antenv.axon_hooks" not in sys.modules:
        mod = types.ModuleType("antenv.axon_hooks")
        state = {"hook": None}
        mod.set_axon_ntff_profile_hook = lambda h: state.__setitem__("hook", h)
        mod.get_axon_ntff_profile_hook = lambda: state["hook"]
        sys.modules["antenv.axon_hooks"] = mod
        try:
            import antenv

            antenv.axon_hooks = mod
            if "/root/.axon_site" not in sys.path:
                sys.path.insert(0, "/root/.axon_site")
            from trn_agent_boot.trn_boot import _ntff_profile_via_ctypes

            mod.set_axon_ntff_profile_hook(
                _ntff_profile_via_ctypes("/opt/axon/libaxon_pjrt.so")
            )
        except Exception:
            pass
    import concourse.bass_utils as bu

    bu.upload_artifacts = lambda tmpdir: tmpdir


def _build_nc():
    import concourse.bass as bass
    import concourse.mybir as mybir
    from concourse.bass import ts
    from concourse.masks import make_identity
    from concourse.tile import TileContext

    f32 = mybir.dt.float32
    bf16 = mybir.dt.bfloat16
    Exp = mybir.ActivationFunctionType.Exp

    nc = bass.Bass()
    x_d = nc.dram_tensor("x", [BPC, S, D], f32, kind="ExternalInput")
    wq_d = nc.dram_tensor("wq", [D, E], f32, kind="ExternalInput")
    wk_d = nc.dram_tensor("wk", [D, E], f32, kind="ExternalInput")
    wv_d = nc.dram_tensor("wv", [D, E], f32, kind="ExternalInput")
    out_d = nc.dram_tensor("out", [BPC, S, E], f32, kind="ExternalOutput")

    with TileContext(nc) as tc:
        with (
            tc.tile_pool(name="consts", bufs=1) as cpool,
            tc.tile_pool(name="xnat", bufs=6) as xpool,
            tc.tile_pool(name="xt", bufs=2) as xtpool,
            tc.tile_pool(name="qt", bufs=2) as qtpool,
            tc.tile_pool(name="kt", bufs=2) as ktpool,
            tc.tile_pool(name="vaug", bufs=2) as vpool,
            tc.tile_pool(name="pt", bufs=4) as ptpool,
            tc.tile_pool(name="otile", bufs=6) as opool,
            tc.tile_pool(name="ps", bufs=2, space="PSUM") as pspool,
            tc.tile_pool(name="scA", bufs=1, space="PSUM") as scApool,
            tc.tile_pool(name="scB", bufs=1, space="PSUM") as scBpool,
            tc.tile_pool(name="accA", bufs=1, space="PSUM") as accApool,
            tc.tile_pool(name="accB", bufs=1, space="PSUM") as accBpool,
        ):
            scpools = [scApool, scBpool]
            accpools = [accApool, accBpool]

            identb = cpool.tile([128, 128], bf16, tag="identb")
            make_identity(nc, identb[:])

            # Wq/Wk packed side-by-side per d-chunk: chunk c occupies cols
            # [128c, 128c+64) = Wq, [128c+64, 128c+128) = Wk.  One [128,128]
            # stationary then projects Q^T and K^T in a single matmul stream.
            wstg = cpool.tile([128, 2 * E], f32, tag="wstg")
            wqk_sb = cpool.tile([128, 3 * 128], bf16, tag="wqk")
            wv_sb = cpool.tile([128, 3 * E], bf16, tag="wv")
            for c in range(3):
                nc.sync.dma_start(wstg[:, 0:E], wq_d[ts(c, 128), :])
                nc.sync.dma_start(wstg[:, E : 2 * E], wk_d[ts(c, 128), :])
                nc.vector.tensor_copy(
                    wqk_sb[:, 128 * c : 128 * c + 128], wstg[:]
                )
            for c in range(3):
                nc.sync.dma_start(wstg[:, 0:E], wv_d[ts(c, 128), :])
                nc.vector.tensor_copy(wv_sb[:, ts(c, E)], wstg[:, 0:E])

            state = {}
            for b in range(BPC):
                st = state[b] = {}
                st["xt"] = xtpool.tile(
                    [128, 3 * S], bf16, tag="xt", name=f"xt_{b}"
                )
                st["qt"] = qtpool.tile([64, S], bf16, tag="qt", name=f"qt_{b}")
                st["kt"] = ktpool.tile([64, S], bf16, tag="kt", name=f"kt_{b}")
                st["va"] = vpool.tile(
                    [128, NB * (E + 1)], bf16, tag="va", name=f"va_{b}"
                )
                st["xn"] = {}

            def xts(b, c, lo, width):
                xt_all = state[b]["xt"]
                return xt_all[:, c * S + lo : c * S + lo + width]

            # ---------- phase-A step emitters (one call = one filler step) ----

            def emit_dma(b, t0, t1):
                """One batched DMA for x tiles [t0, t1): row 128a+p lands on
                partition p at free offset a*D, so each 128-row tile is a
                contiguous [128, D] slice."""
                nt = t1 - t0
                xn = xpool.tile(
                    [128, 4 * D], f32, tag="xn", name=f"xn_{b}_{t0}"
                )
                nc.sync.dma_start(
                    xn[:, 0 : nt * D].rearrange("p (a d) -> p a d", a=nt),
                    x_d[b, 128 * t0 : 128 * t1, :].rearrange(
                        "(a p) d -> p a d", p=128
                    ),
                )
                for t in range(t0, t1):
                    state[b]["xn"][t] = (xn, (t - t0) * D)

            def emit_tr(b, t):
                """Cast x tile t to bf16 (ACT for batch 0, DVE for batch 1 —
                ACT is idle during the load phase), PE-transpose it, and copy
                back PSUM->SBUF."""
                xn, off = state[b]["xn"].pop(t)
                xb = xpool.tile([128, D], bf16, tag="xb", name=f"xb_{b}_{t}")
                if b == 0:
                    nc.scalar.copy(xb[:], xn[:, off : off + D])
                else:
                    nc.vector.tensor_copy(xb[:], xn[:, off : off + D])
                pxt = pspool.tile([128, 512], bf16, tag="ps", name=f"pxt_{b}_{t}")
                for c in range(3):
                    nc.tensor.transpose(
                        pxt[:, 128 * c : 128 * c + 128],
                        xb[:, ts(c, 128)],
                        identb[:],
                    )
                xt3 = state[b]["xt"][:].rearrange("p (c s) -> p c s", c=3)
                nc.vector.tensor_copy(
                    xt3[:, :, ts(t, 128)],
                    pxt[:, 0:D].rearrange("p (c s) -> p c s", c=3),
                )

            def emit_qk(b, n):
                """Project Q^T/K^T for sequence chunk n (512 cols).  The kt
                copy gates the next panel's first score LDWEIGHTS; for early
                chunks the qt copy rides on the then-idle ACT engine so kt
                starts on DVE immediately."""
                qt, kt = state[b]["qt"], state[b]["kt"]
                pq = pspool.tile([128, 512], f32, tag="ps", name=f"pq_{b}_{n}")
                for c in range(3):
                    nc.tensor.matmul(
                        pq[:],
                        wqk_sb[:, ts(c, 128)],
                        xts(b, c, 512 * n, 512),
                        start=(c == 0),
                        stop=(c == 2),
                    )
                if n <= 1:
                    nc.scalar.copy(qt[:, ts(n, 512)], pq[:64, :])
                else:
                    nc.vector.tensor_copy(qt[:, ts(n, 512)], pq[:64, :])
                nc.vector.tensor_copy(kt[:, ts(n, 512)], pq[64:128, :])

            def emit_v(b, g):
                """Project V for key blocks 4g..4g+3 and set ones column."""
                va3 = state[b]["va"][:].rearrange("p (k e) -> p k e", k=NB)
                pv = pspool.tile([128, 512], f32, tag="ps", name=f"pv_{b}_{g}")
                for j in range(4):
                    k = 4 * g + j
                    for c in range(3):
                        nc.tensor.matmul(
                            pv[:, 64 * j : 64 * j + 64],
                            xts(b, c, 128 * k, 128),
                            wv_sb[:, ts(c, E)],
                            start=(c == 0),
                            stop=(c == 2),
                        )
                nc.vector.tensor_copy(
                    va3[:, 4 * g : 4 * g + 4, 0:E],
                    pv[:, 0:256].rearrange("p (k e) -> p k e", k=4),
                )
                nc.gpsimd.memset(va3[:, 4 * g : 4 * g + 4, E : E + 1], 1.0)

            # ---------- attention ------------------------------------------

            # HAM pacing: every PACE-th round both streams emit their PV
            # immediately (PE waits on its own exp), capping the PE's
            # per-epoch activity below the hardware throttle trigger.
            PACE = 3
            serialize = [False]

            def gen_attn(b):
                """Yields once per exp-unit (block pair or diagonal single)
                and once per panel epilogue.  PV emission is delayed one
                yield so exp() of the current unit overlaps the other
                stream's matmuls.  PV accumulates in natural [q, e'] layout:
                lhsT = P^T piece (stationary), rhs = V_aug block."""
                qt, kt, va_all = state[b]["qt"], state[b]["kt"], state[b]["va"]
                scpool, accpool = scpools[b], accpools[b]
                for h in range(NPAN):
                    base = H * h
                    nblk = 4 * h + 4
                    # acc[q=128, j*65 + e'] for the panel's 4 q-sub-blocks
                    acc = accpool.tile([128, 4 * (E + 1)], f32, tag=f"acc{b}")

                    def emit_pv(blocks, acc=acc, base=base, h=h):
                        for (i, pt_ap, qlo) in blocks:
                            # pieces: q sub-blocks j covered by key block i
                            j0 = max(0, (qlo - base) // 128)
                            for j in range(j0, 4):
                                qq = base + 128 * j
                                # one start per bank: the pending-zero
                                # region it opens initializes each q
                                # sub-block's first write
                                nc.tensor.matmul(
                                    acc[:, 65 * j : 65 * j + 65],
                                    pt_ap[:, qq - qlo : qq - qlo + 128],
                                    va_all[:, 65 * i : 65 * i + 65],
                                    start=(i == 0 and j == 0),
                                    stop=(i == 4 * h + j),
                                    skip_group_check=True,
                                )

                    pending = None
                    # full-width pairs (blocks 0..4h-1)
                    for j in range(h):
                        for half in range(2):
                            i0 = 4 * j + 2 * half
                            ps = scpool.tile([128, 2 * 512], f32, tag=f"sc{b}")
                            for u in range(2):
                                nc.tensor.matmul(
                                    ps[:, 512 * u : 512 * u + 512],
                                    kt[:, ts(i0 + u, 128)],
                                    qt[:, base : base + 512],
                                    start=True,
                                    stop=True,
                                )
                            pt = ptpool.tile([128, 1024], bf16, tag="pt")
                            nc.scalar.activation(pt[:], ps[:], Exp, scale=SCALE)
                            if pending is not None:
                                emit_pv(pending)
                            pending = [
                                (i0, pt[:, 0:512], base),
                                (i0 + 1, pt[:, 512:1024], base),
                            ]
                            if serialize[0]:
                                emit_pv(pending)
                                pending = None
                            yield
                    # diagonal singles (blocks 4h..4h+3), two per exp: the
                    # second single lands at col 512 (its own PSUM bank);
                    # the exp covers the gap cols too (junk, never read)
                    for i in range(4 * h, nblk, 2):
                        qlo0, qlo1 = 128 * i, 128 * (i + 1)
                        w0 = base + H - qlo0
                        w1 = base + H - qlo1
                        # second single packs right after the first when both
                        # fit in one PSUM bank, else at the bank boundary
                        off1 = w0 if w0 + w1 <= 512 else 512
                        ps = scpool.tile([128, 2 * 512], f32, tag=f"sc{b}")
                        nc.tensor.matmul(
                            ps[:, 0:w0],
                            kt[:, ts(i, 128)],
                            qt[:, qlo0 : qlo0 + w0],
                            start=True,
                            stop=True,
                        )
                        nc.tensor.matmul(
                            ps[:, off1 : off1 + w1],
                            kt[:, ts(i + 1, 128)],
                            qt[:, qlo1 : qlo1 + w1],
                            start=True,
                            stop=True,
                        )
                        pt = ptpool.tile([128, 1024], bf16, tag="pt")
                        nc.scalar.activation(
                            pt[:, 0 : off1 + w1], ps[:, 0 : off1 + w1], Exp, scale=SCALE
                        )
                        for off in (0, off1):
                            nc.gpsimd.affine_select(
                                out=pt[:, off : off + 128],
                                in_=pt[:, off : off + 128],
                                compare_op=mybir.AluOpType.is_ge,
                                fill=0.0,
                                base=0,
                                pattern=[[1, 128]],
                                channel_multiplier=-1,
                            )
                        if pending is not None:
                            emit_pv(pending)
                        pending = [
                            (i, pt[:, 0:w0], qlo0),
                            (i + 1, pt[:, off1 : off1 + w1], qlo1),
                        ]
                        if serialize[0]:
                            emit_pv(pending)
                            pending = None
                        yield
                    if pending is not None:
                        emit_pv(pending)
                    # epilogue: strided reciprocal of the 4 denominator
                    # columns, one broadcast multiply, one DMA
                    acc4 = acc[:].rearrange("p (a b) -> p a b", a=4)
                    rc4 = opool.tile([128, 4], f32, tag="rc")
                    nc.vector.reciprocal(
                        rc4[:].rearrange("p (a b) -> p a b", b=1),
                        acc4[:, :, E : E + 1],
                    )
                    ot4 = opool.tile([128, 4 * E], f32, tag="ot")
                    ot4v = ot4[:].rearrange("p (a e) -> p a e", a=4)
                    nc.vector.tensor_mul(
                        ot4v,
                        acc4[:, :, 0:E],
                        rc4[:]
                        .rearrange("p (a b) -> p a b", b=1)
                        .broadcast_to([128, 4, E]),
                    )
                    nc.sync.dma_start(
                        out_d[b, H * h : H * h + H, :].rearrange(
                            "(a s) e -> s a e", a=4
                        ),
                        ot4v,
                    )
                    yield

            # ---------- schedule -------------------------------------------

            # staggered prologue: batch A's panel-0 pipeline first so its
            # attention starts as early as possible; batch B's panel-0 work
            # leads the filler and gates B's first attention step.  x DMA
            # issue order (A-priority) is aligned with the round-robin's
            # consumption order: A01 A23 A4567 B0123 | A8-11 B4567 |
            # A12-15 B8-11 | B12-15.
            emit_dma(0, 0, 2)
            emit_dma(0, 2, 4)
            emit_dma(0, 4, 8)
            emit_dma(1, 0, 4)
            for t in range(4):
                emit_tr(0, t)
            emit_qk(0, 0)
            emit_v(0, 0)

            fill = []
            marker = {}  # (b, p) -> fill index that must be drained
            for t in range(4):
                fill.append(lambda t=t: emit_tr(1, t))
            fill.append(lambda: emit_qk(1, 0))
            fill.append(lambda: emit_v(1, 0))
            marker[(1, 0)] = len(fill)
            for p in range(1, NPAN):
                # batch A group: prefetch A's next tiles, transpose own,
                # project, then release B's tiles for this panel
                if p + 1 < NPAN:
                    fill.append(
                        lambda p=p: emit_dma(0, 4 * (p + 1), 4 * (p + 1) + 4)
                    )
                for t in range(4 * p, 4 * p + 4):
                    fill.append(lambda t=t: emit_tr(0, t))
                fill.append(lambda p=p: emit_qk(0, p))
                fill.append(lambda p=p: emit_v(0, p))
                fill.append(lambda p=p: emit_dma(1, 4 * p, 4 * p + 4))
                marker[(0, p)] = len(fill)
                # batch B group
                for t in range(4 * p, 4 * p + 4):
                    fill.append(lambda t=t: emit_tr(1, t))
                fill.append(lambda p=p: emit_qk(1, p))
                fill.append(lambda p=p: emit_v(1, p))
                marker[(1, p)] = len(fill)

            fill_i = 0

            def drain_to(idx):
                nonlocal fill_i
                while fill_i < idx:
                    fill[fill_i]()
                    fill_i += 1

            def emit_filler(k):
                nonlocal fill_i
                n = min(k, len(fill) - fill_i)
                for _ in range(n):
                    fill[fill_i]()
                    fill_i += 1

            # panel start positions in yield index space: panel h has
            # 2h+2 block-units + 1 epilogue yield
            panel_start = {}
            accum = 0
            for h in range(NPAN):
                panel_start[h] = accum
                accum += 2 * h + 3

            gens = [gen_attn(0), gen_attn(1)]
            alive = [True, True]
            ycount = [0, 0]
            step = 0
            rnd = 0
            while any(alive):
                serialize[0] = rnd % PACE == PACE - 1
                rnd += 1
                for gi in range(2):
                    if not alive[gi]:
                        continue
                    # before entering panel p, its phase prereqs must be in
                    for h in range(NPAN):
                        if ycount[gi] == panel_start[h] and (gi, h) in marker:
                            drain_to(marker[(gi, h)])
                    try:
                        next(gens[gi])
                        ycount[gi] += 1
                    except StopIteration:
                        alive[gi] = False
                    # serialize rounds emit no filler so the pacing gap is
                    # real PE idle, not filled by phase work
                    if not serialize[0]:
                        emit_filler(2 if step % 2 == 0 else 1)
                    step += 1
            drain_to(len(fill))

    _split_multi_waits(nc)
    return nc


def _get_nc():
    if "nc" not in _cache:
        _install_patches()
        _cache["nc"] = _build_nc()
    return _cache["nc"]


def kernel(x, Wq, Wk, Wv):
    from concourse.bass_utils import run_bass_kernel_spmd

    nc = _get_nc()
    x = np.ascontiguousarray(x, dtype=np.float32)
    in_maps = [
        {
            "x": x[i * BPC : (i + 1) * BPC],
            "wq": np.asarray(Wq, dtype=np.float32),
            "wk": np.asarray(Wk, dtype=np.float32),
            "wv": np.asarray(Wv, dtype=np.float32),
        }
        for i in range(N_CORES)
    ]
    res = run_bass_kernel_spmd(nc, in_maps, list(range(N_CORES)))
    out = np.concatenate([res.results[i]["out"] for i in range(N_CORES)], axis=0)
    return out.astype(np.float32)
